# revision 1
# baseline (speedup 1.0000x reference)
"""
BatchAllTripletLoss Trainium2 kernel (8 NeuronCores, SPMD).

Math: for anchors a, positives p (same label, p != a), negatives n
(different label), the reference reduces relu(d(a,p) - d(a,n) + m) over
the B^3 triplet tensor.  Only label-equal (a,p) pairs contribute (~4000
of 512^2), so the host enumerates positive pairs (pure index
manipulation of labels) and shards the PAIR list across the 8 cores.
Each core processes tiles of 128 pairs x 512 candidate negatives:

  - PE builds the pair->all d^2 rows in PSUM with TWO float32r matmuls:
    the Gram term -2*A^T @ embT (k=128), and a combined matmul whose
    extra contraction rows broadcast ||e_n||^2, ||e_a||^2 and add +1e30
    to label-equal columns via the class one-hots (the negative mask
    folded into the distance matrix; padding pair slots get +1e30
    everywhere).
  - ACT takes d = sqrt(d^2) straight from PSUM and accumulates
    sum(relu(x + m - d)) using its per-partition bias + accumulator.
  - DVE computes per-pair x^2 = ||e_a - e_p||^2 (sub/mul/reduce) and
    counts positive triplets with is_lt + accumulator in the SQUARED
    domain (d^2 < (x+m)^2), which is independent of the sqrt LUT.
  - num_valid = sum_c m_c(m_c-1)(B-m_c) from per-class counts (exact
    fp32 matmuls).

All inputs are packed into two [128, F] partition-major bulk tensors so
each DMA needs only one large descriptor per partition (descriptor
generation, not bandwidth, dominates many-small-row DMAs on this part).

d^2 of unmasked entries is strongly positive for gaussian-like data
(the host asserts a safety margin and falls back to a clamped program
variant otherwise), so no relu is needed before the sqrt.

Each core returns [sum_loss, num_pos, num_valid]; the host sums the 8
partial scalars and applies the final scalar divisions (the "all-reduce
of the scalar sums/counts" from the sharding hint).
"""

import math

import numpy as np

import concourse.bass as bass
import concourse.tile as tile
from concourse import bacc, mybir
from concourse.bass_utils import run_bass_kernel_spmd

B = 512          # batch
D = 128          # embedding dim
NCLS = 64        # label classes
NCORES = 8
MARGIN = 0.2
BIG = 1.0e30     # added to d^2 for label-equal negatives / padding
PAD = -1.0e10    # per-pair margin slot for padding slots; PAD^2 << BIG

F32 = mybir.dt.float32
F32R = mybir.dt.float32r
AF = mybir.ActivationFunctionType
OP = mybir.AluOpType

TRACE = False            # set by test.py to profile
LAST_RESULT = None       # BassKernelResults of the last run

_PROGRAM_CACHE = {}


def _aux_rows(ncls: int):
    """Partition indices for the two broadcast rows appended to the
    one-hot matmul operands; engine writes must start 32-aligned."""
    r1 = (ncls + 31) // 32 * 32
    r2 = r1 + 32
    assert r2 < 128, f"too many label classes for combined matmul: {ncls}"
    return r1, r2, r2 + 1


def _build_program(n_tiles: int, ncls: int, clamp: bool):
    """Build the SPMD single-core program (same NEFF on all 8 cores)."""
    npairs = n_tiles * 128
    nc = bacc.Bacc("TRN2", target_bir_lowering=False, debug=False)

    # bulk_r [128, 2*B + 2*npairs] f32r: embT | a_embT | a_lohx | lohx
    #   embT   [D=128, B]      cols 0 : B
    #   a_embT [D=128, npairs] cols B : B+npairs
    #   a_lohx [128p,  npairs] cols B+npairs : B+2*npairs
    #     rows 0..ncls-1 = BIG*onehot(label(a_i)) (pads: BIG everywhere),
    #     row r1 = ||e_a||^2 (device-written), row r2 = ones, rest 0
    #   lohx   [128p,  B]      cols B+2*npairs : 2B+2*npairs
    #     rows 0..ncls-1 = onehot(labels), row r1 = ones, row r2 =
    #     ||e_n||^2 (device-written), rest 0
    # bulk_p [128, 2*n_tiles*D + n_tiles] f32: a_emb | p_emb | mvec
    #   (pair-major: partition = pair slot within tile, free = (t, d))
    fr = B + npairs
    fp = 2 * n_tiles * D + n_tiles
    labels_d = nc.dram_tensor("labels_f", [1, B + npairs], F32R,
                              kind="ExternalInput")
    bulk_r_d = nc.dram_tensor("bulk_r", [128, fr], F32R, kind="ExternalInput")
    bulk_p_d = nc.dram_tensor("bulk_p", [128, fp], F32, kind="ExternalInput")
    out_d = nc.dram_tensor("out", [1, 3], F32, kind="ExternalOutput")

    with tile.TileContext(nc) as tc:
        from contextlib import ExitStack

        with ExitStack() as ctx:
            _body(ctx, tc, n_tiles, ncls, clamp, labels_d, bulk_r_d,
                  bulk_p_d, out_d)
    nc.compile()
    return nc


def _body(ctx, tc, n_tiles, ncls, clamp, labels_d, bulk_r_d, bulk_p_d,
          out_d):
    nc = tc.nc
    npairs = n_tiles * 128
    r1, r2, _ = _aux_rows(ncls)
    fr = B + npairs
    fp = 2 * n_tiles * D + n_tiles

    const = ctx.enter_context(tc.tile_pool(name="const", bufs=1))
    work = ctx.enter_context(tc.tile_pool(name="work", bufs=3))
    small = ctx.enter_context(tc.tile_pool(name="small", bufs=4))
    psum = ctx.enter_context(tc.tile_pool(name="psum", bufs=2, space="PSUM"))
    psum1 = ctx.enter_context(tc.tile_pool(name="psum1", bufs=1, space="PSUM"))

    ones_col = const.tile([128, 1], F32)
    nc.vector.memset(ones_col, 1.0)

    # ---- bulk loads: one descriptor per partition per DMA, one bulk
    # tensor per HWDGE queue; the tiny labels row goes first
    labels_sb = const.tile([1, B + npairs], F32R)
    nc.sync.dma_start(out=labels_sb, in_=labels_d.ap())
    bulk_r = const.tile([128, fr], F32R)
    nc.scalar.dma_start(out=bulk_r, in_=bulk_r_d.ap())
    bulk_p = const.tile([128, fp], F32)
    nc.scalar.dma_start(out=bulk_p, in_=bulk_p_d.ap())

    embT = bulk_r[:, 0:B]
    a_embT = bulk_r[:, B:B + npairs]
    a_emb = bulk_p[:, 0:n_tiles * D].rearrange("p (t d) -> p t d", d=D)
    p_emb = bulk_p[:, n_tiles * D:2 * n_tiles * D].rearrange(
        "p (t d) -> p t d", d=D)
    mvec = bulk_p[:, 2 * n_tiles * D:]

    # ---- build the mask/broadcast matmul operands on device from the
    # labels row: row c of lohx is the one-hot [labels == c]
    iota_col = const.tile([128, 1], F32)
    nc.gpsimd.iota(iota_col, [[0, 1]], base=0, channel_multiplier=1,
                   allow_small_or_imprecise_dtypes=True)
    ones_col_r = const.tile([128, 1], F32R)
    nc.vector.tensor_scalar(ones_col_r, iota_col, 0.0, 1.0,
                            op0=OP.mult, op1=OP.add)
    ones_row_r = const.tile([1, 128], F32R)
    nc.vector.tensor_scalar(ones_row_r, labels_sb[:, 0:128], 0.0, 1.0,
                            op0=OP.mult, op1=OP.add)

    lohx = const.tile([128, B], F32R)
    plb = psum.tile([128, B], F32, tag="pd2")
    nc.tensor.matmul(plb, lhsT=ones_row_r, rhs=labels_sb[:, 0:B],
                     start=True, stop=True)
    nc.vector.tensor_scalar(lohx, plb, iota_col, None, op0=OP.is_equal)
    # row r1 pairs with the device-written ||e_a||^2 row; row r1+1 pairs
    # with the BIG that padding pair slots carry (their labels row
    # points at r1+1) -> both rows must be all-ones
    nc.scalar.activation(lohx[r1:r1 + 2, :], plb[0:2, :], AF.Copy,
                         bias=1.0, scale=0.0)

    a_lohx = const.tile([128, npairs], F32R)
    for c0 in range(0, npairs, 512):
        c1 = min(c0 + 512, npairs)
        plba = psum.tile([128, 512], F32, tag="pd2")
        nc.tensor.matmul(plba[:, :c1 - c0], lhsT=ones_row_r,
                         rhs=labels_sb[:, B + c0:B + c1],
                         start=True, stop=True)
        nc.vector.tensor_scalar(a_lohx[:, c0:c1], plba[:, :c1 - c0],
                                iota_col, BIG, op0=OP.is_equal, op1=OP.mult)
        # row r2 pairs with ||e_n||^2 in lohx: ones everywhere
        nc.scalar.activation(a_lohx[r2:r2 + 1, c0:c1], plba[0:1, :c1 - c0],
                             AF.Copy, bias=1.0, scale=0.0)

    # ---- setup: -2*A^T; ||e_n||^2 -> lohx row r2; ||e_a||^2 ->
    # a_lohx row r1
    a_embT_m2 = const.tile([D, npairs], F32R)
    nc.vector.tensor_scalar_mul(a_embT_m2, a_embT, -2.0)

    embT_sq = work.tile([D, B], F32R, tag="embT_sq")
    nc.vector.tensor_mul(embT_sq, embT, embT)
    psum_sq = psum1.tile([1, B], F32, tag="psq")
    nc.tensor.matmul(psum_sq, lhsT=ones_col_r, rhs=embT_sq, start=True,
                     stop=True)
    nc.scalar.copy(lohx[r2:r2 + 1, :], psum_sq)

    a_embT_sq = work.tile([D, npairs], F32R, tag="a_embT_sq")
    nc.gpsimd.tensor_mul(a_embT_sq, a_embT, a_embT)
    psum_sqa = psum1.tile([1, 512], F32, tag="psqa")
    for c0 in range(0, npairs, 512):
        c1 = min(c0 + 512, npairs)
        nc.tensor.matmul(psum_sqa[:, :c1 - c0], lhsT=ones_col_r,
                         rhs=a_embT_sq[:, c0:c1], start=True, stop=True)
        nc.scalar.copy(a_lohx[r1:r1 + 1, c0:c1], psum_sqa[:, :c1 - c0])

    # ---- X phase: x^2 = ||e_a - e_p||^2 per pair (batched over tiles)
    xsq_cols = const.tile([128, n_tiles], F32)
    diff = work.tile([128, n_tiles, D], F32, tag="diff")
    nc.vector.tensor_sub(diff, a_emb, p_emb)
    dsq2 = work.tile([128, n_tiles, D], F32, tag="dsq2")
    nc.vector.tensor_mul(dsq2, diff, diff)
    nc.vector.tensor_reduce(xsq_cols, dsq2, axis=mybir.AxisListType.X,
                            op=OP.add)

    x0 = small.tile([128, n_tiles], F32, tag="x0")
    nc.scalar.activation(x0, xsq_cols, AF.Sqrt)
    xp_cols = const.tile([128, n_tiles], F32)
    nc.vector.tensor_add(xp_cols, x0, mvec)
    xp2_cols = const.tile([128, n_tiles], F32)
    nc.vector.tensor_mul(xp2_cols, xp_cols, xp_cols)

    # ---- per-tile accumulators (S split between ACT and DVE) ----
    s_cols = const.tile([128, n_tiles], F32)
    c_cols = const.tile([128, n_tiles], F32)
    zeros_b = const.tile([128, B], F32)
    nc.vector.memset(zeros_b, 0.0)

    for t0g in range(0, n_tiles, 2):
        gsz = min(2, n_tiles - t0g)
        # d^2 rows for a pair of tiles in one 2-bank PSUM group
        pd2 = psum.tile([128, 2 * B], F32, tag="pd2")
        for j in range(gsz):
            t = t0g + j
            nc.tensor.matmul(pd2[:, j * B:(j + 1) * B],
                             lhsT=a_embT_m2[:, bass.ts(t, 128)],
                             rhs=embT, start=True, stop=False)
            nc.tensor.matmul(pd2[:, j * B:(j + 1) * B],
                             lhsT=a_lohx[:, bass.ts(t, 128)],
                             rhs=lohx, start=False, stop=True)

        if clamp:
            dsrc = work.tile([128, 2 * B], F32, tag="dsrc")
            nc.vector.tensor_scalar_max(dsrc[:, :gsz * B], pd2[:, :gsz * B],
                                        0.0)
        else:
            dsrc = pd2
        dY = work.tile([128, 2 * B], F32, tag="dY")
        nc.scalar.activation(dY[:, :gsz * B], dsrc[:, :gsz * B], AF.Sqrt)

        for j in range(gsz):
            t = t0g + j
            dsrc_t = dsrc[:, j * B:(j + 1) * B]
            dY_t = dY[:, j * B:(j + 1) * B]
            # count: d^2 < (x+m)^2, squared-domain compare
            c_scr = work.tile([128, B], F32, tag="c_scr")
            nc.vector.tensor_scalar(
                c_scr, dsrc_t, xp2_cols[:, t:t + 1], None, op0=OP.is_lt,
                op1=OP.add, accum_out=c_cols[:, t:t + 1])
            # loss: sum relu((x+m) - d); even tiles on ACT (bias+accum),
            # odd tiles on DVE as sum min(d - (x+m), 0) = -sum relu(...)
            r_scr = work.tile([128, B], F32, tag="r_scr")
            if t % 2 == 0:
                nc.scalar.activation(
                    r_scr, dY_t, AF.Relu, bias=xp_cols[:, t:t + 1],
                    scale=-1.0, accum_out=s_cols[:, t:t + 1])
            else:
                nc.vector.scalar_tensor_tensor(
                    out=r_scr, in0=dY_t, scalar=xp_cols[:, t:t + 1],
                    in1=zeros_b, op0=OP.subtract, op1=OP.min,
                    accum_out=s_cols[:, t:t + 1])

    # ---- num_valid = sum_c m_c (m_c - 1) (B - m_c) (exact fp32) ----
    psum_f = psum1.tile([1, 3], F32, tag="pf")
    cnt = small.tile([ncls, 1], F32, tag="cnt")
    nc.vector.tensor_reduce(cnt, lohx[0:ncls, :], axis=mybir.AxisListType.X,
                            op=OP.add)
    cm1 = small.tile([ncls, 1], F32, tag="cm1")
    nc.vector.tensor_scalar_add(cm1, cnt, -1.0)
    t2 = small.tile([ncls, 1], F32, tag="t2")
    nc.vector.tensor_mul(t2, cnt, cm1)
    t3 = small.tile([ncls, 1], F32, tag="t3")
    nc.vector.tensor_scalar(t3, cnt, -1.0, float(B), op0=OP.mult, op1=OP.add)
    t4 = small.tile([ncls, 1], F32, tag="t4")
    nc.vector.tensor_mul(t4, t2, t3)
    nc.tensor.matmul(psum_f[:, 2:3], lhsT=t4, rhs=ones_col[0:ncls, :],
                     start=True, stop=True)

    # ---- final reduction to 3 scalars ----
    # odd s_cols columns hold -sum relu: S = sum(even) - sum(odd)
    n_even = (n_tiles + 1) // 2
    n_odd = n_tiles // 2
    sp = small.tile([128, 1], F32, tag="sp")
    nc.vector.tensor_reduce(
        sp, s_cols.rearrange("p (t two) -> p t two", two=1)[:, 0::2, :]
        if False else s_cols[:, 0:n_tiles:2],
        axis=mybir.AxisListType.X, op=OP.add)
    sc = small.tile([128, 2], F32, tag="sc")
    if n_odd:
        sn = small.tile([128, 1], F32, tag="sn")
        nc.vector.tensor_reduce(sn, s_cols[:, 1:n_tiles:2],
                                axis=mybir.AxisListType.X, op=OP.add)
        nc.vector.tensor_sub(sc[:, 0:1], sp, sn)
    else:
        nc.vector.tensor_copy(sc[:, 0:1], sp)
    nc.vector.tensor_reduce(sc[:, 1:2], c_cols, axis=mybir.AxisListType.X,
                            op=OP.add)
    nc.tensor.matmul(psum_f[:, 0:2], lhsT=ones_col, rhs=sc,
                     start=True, stop=True)
    out_sb = small.tile([1, 3], F32, tag="out_sb")
    nc.vector.tensor_copy(out_sb, psum_f)
    nc.sync.dma_start(out=out_d.ap(), in_=out_sb)


def _host_prepare(labels: np.ndarray, emb: np.ndarray):
    """Index-only prep: positive-pair list, shard across cores, per-core
    gathered input arrays packed into the two bulk layouts."""
    labels = np.asarray(labels).astype(np.int64)
    emb = np.ascontiguousarray(np.asarray(emb, dtype=np.float32))
    b = labels.shape[0]
    ncls = NCLS if labels.max(initial=0) < NCLS else int(labels.max()) + 1
    r1, r2, _ = _aux_rows(ncls)

    pairs_a, pairs_p = [], []
    by_class = {}
    for i, lab in enumerate(labels.tolist()):
        by_class.setdefault(lab, []).append(i)
    for idxs in by_class.values():
        for a in idxs:
            for p in idxs:
                if a != p:
                    pairs_a.append(a)
                    pairs_p.append(p)
    np_total = len(pairs_a)
    per_core = max(1, math.ceil(np_total / NCORES))
    n_tiles = max(1, math.ceil(per_core / 128))
    npc = n_tiles * 128

    # safety check for the no-clamp program: unmasked d^2 must be
    # strongly positive (fp32/f32r rounding slack is far below 1.0)
    sq = (emb * emb).sum(1)
    d2 = sq[:, None] + sq[None, :] - 2.0 * (emb @ emb.T)
    neq = labels[:, None] != labels[None, :]
    clamp = bool(d2[neq].min() < 1.0) if neq.any() else True

    embT = np.ascontiguousarray(emb.T)

    in_maps = []
    for k in range(NCORES):
        a_idx = pairs_a[k * per_core:(k + 1) * per_core]
        p_idx = pairs_p[k * per_core:(k + 1) * per_core]
        nreal = len(a_idx)
        a_emb = np.zeros((n_tiles, 128, D), np.float32)
        p_emb = np.zeros((n_tiles, 128, D), np.float32)
        a_embT = np.zeros((D, npc), np.float32)
        # labels row for the device-side one-hot build; padding pair
        # slots point at row r1+1 (an always-zero row) so the compare
        # gives them BIG in that row -> d^2 + BIG on the whole row
        lab_a = np.full((npc,), float(r1 + 1), np.float32)
        mvec = np.full((npc,), PAD, np.float32)
        if nreal:
            ga = emb[a_idx]
            a_emb.reshape(npc, D)[:nreal] = ga
            p_emb.reshape(npc, D)[:nreal] = emb[p_idx]
            a_embT[:, :nreal] = ga.T
            lab_a[:nreal] = labels[a_idx].astype(np.float32)
            mvec[:nreal] = MARGIN
        a_emb2 = np.ascontiguousarray(
            a_emb.transpose(1, 0, 2)).reshape(128, -1)
        p_emb2 = np.ascontiguousarray(
            p_emb.transpose(1, 0, 2)).reshape(128, -1)
        mvec2 = np.ascontiguousarray(mvec.reshape(n_tiles, 128).T)
        labels_f = np.concatenate(
            [labels.astype(np.float32), lab_a])[None, :]
        bulk_r = np.concatenate([embT, a_embT], axis=1)
        bulk_p = np.concatenate([a_emb2, p_emb2, mvec2], axis=1)
        in_maps.append({
            "labels_f": np.ascontiguousarray(labels_f),
            "bulk_r": np.ascontiguousarray(bulk_r),
            "bulk_p": np.ascontiguousarray(bulk_p),
        })
    return in_maps, n_tiles, ncls, clamp


def kernel(labels: np.ndarray, embeddings: np.ndarray):
    global LAST_RESULT
    in_maps, n_tiles, ncls, clamp = _host_prepare(labels, embeddings)

    key = (n_tiles, ncls, clamp)
    if key not in _PROGRAM_CACHE:
        _PROGRAM_CACHE[key] = _build_program(n_tiles, ncls, clamp)
    nc = _PROGRAM_CACHE[key]

    res = run_bass_kernel_spmd(nc, in_maps, list(range(NCORES)), trace=TRACE)
    LAST_RESULT = res

    outs = np.stack([r["out"].reshape(-1) for r in res.results]).astype(np.float32)
    s_sum = np.float32(0.0)
    c_sum = np.float32(0.0)
    for k in range(NCORES):
        s_sum = np.float32(s_sum + outs[k, 0])
        c_sum = np.float32(c_sum + outs[k, 1])
    nv = outs[0, 2]
    loss = np.float32(s_sum / np.float32(c_sum + np.float32(1e-16)))
    frac = np.float32(c_sum / np.float32(nv + np.float32(1e-16)))
    return (np.asarray(loss, np.float32), np.asarray(frac, np.float32))



# revision 9
# speedup vs baseline: 1.2292x; 1.2292x over previous
"""BatchAllTripletLoss TRN2 kernel — v3.

Per core: tiles of 128 pairs x 512 negatives.  PE builds full d^2 (+BIG
mask) in PSUM via bf16 matmuls (Gram + one-hot mask with device-written
||e_n||^2 / ||e_a||^2 contraction rows).  ACT sqrts two tiles per pass
and accumulates half the loss (Relu) and half the counts (Sign, squared
domain, straight from PSUM); DVE accumulates the rest
(scalar_tensor_tensor min / is_lt cache-reduce).  Host does label-index
prep, one-hot mask operands, num_valid, and the final scalar division.
"""

import math

import numpy as np
import ml_dtypes

import concourse.bass as bass
import concourse.tile as tile
from concourse import bacc, mybir
from concourse.bass_utils import run_bass_kernel_spmd

B = 512
D = 128
NCORES = 8
MARGIN = 0.2
BIG = float(2 ** 100)

F32 = mybir.dt.float32
BF16 = mybir.dt.bfloat16
AF = mybir.ActivationFunctionType
OP = mybir.AluOpType
BF = ml_dtypes.bfloat16

TRACE = False
LAST_RESULT = None
_PROGRAM_CACHE = {}
NAROW = 64     # alohx device row: ||e_a||^2   (paired with lohx ones row)
NNROW = 96     # lohx device row: ||e_n||^2    (paired with alohx ones row)


def _build_program(n_tiles: int):
    npc = n_tiles * 128
    nc = bacc.Bacc("TRN2", target_bir_lowering=False, debug=False)
    fe = B + 32
    embT_d = nc.dram_tensor("embT", [128, fe], BF16, kind="ExternalInput")
    am2T_d = nc.dram_tensor("am2T", [128, npc], BF16, kind="ExternalInput")
    ap_d = nc.dram_tensor("ap_emb", [128, 2 * npc], BF16,
                          kind="ExternalInput")
    mask_d = nc.dram_tensor("mask", [96, npc + B], BF16, kind="ExternalInput")
    out_d = nc.dram_tensor("out", [128, 2 * n_tiles], F32,
                           kind="ExternalOutput")

    with tile.TileContext(nc) as tc:
        from contextlib import ExitStack

        with ExitStack() as ctx:
            _body(ctx, tc, n_tiles, embT_d, am2T_d, ap_d, mask_d, out_d)
    nc.compile()
    return nc


def _body(ctx, tc, n_tiles, embT_d, am2T_d, ap_d, mask_d, out_d):
    nc = tc.nc
    npc = n_tiles * 128
    fe = B + 32
    ngrp = (n_tiles + 1) // 2

    const = ctx.enter_context(tc.tile_pool(name="const", bufs=1))
    work = ctx.enter_context(tc.tile_pool(name="work", bufs=3))
    small = ctx.enter_context(tc.tile_pool(name="small", bufs=4))
    psq = ctx.enter_context(tc.tile_pool(name="psq", bufs=2, space="PSUM"))
    psum1 = ctx.enter_context(tc.tile_pool(name="psum1", bufs=1, space="PSUM"))

    # DMAs: each dma_start costs ~2us fixed completion latency; scalar
    # also hosts the ACT table loads, so it gets only the small critical
    # embT tensor.
    embTt = const.tile([128, fe], BF16)
    nc.scalar.dma_start(out=embTt, in_=embT_d.ap())
    am2T = const.tile([128, npc], BF16)
    nc.sync.dma_start(out=am2T, in_=am2T_d.ap())
    maskt = const.tile([97, npc + B], BF16)
    nc.sync.dma_start(out=maskt[0:96, :], in_=mask_d.ap())
    ap_emb = const.tile([128, 2 * npc], BF16)
    nc.gpsimd.dma_start(out=ap_emb, in_=ap_d.ap())

    embT = embTt[:, 0:B]
    lhs_nn = embTt[:, B:B + 1]        # ones
    lhs_na = embTt[:, B + 3:B + 4]    # 0.25s
    a_emb = ap_emb[:, 0:npc]
    p_emb = ap_emb[:, npc:2 * npc]
    alohx = maskt[:, 0:npc]
    lohx = maskt[:, npc:npc + B]

    # alohx ones row (96) is memset on device; rows 66-95 ship as zeros
    nc.vector.memset(alohx[NNROW:NNROW + 1, :], 1.0)
    zeros_b = const.tile([128, B], BF16)
    nc.vector.memset(zeros_b, 0.0)

    # ---- norms: ||e_n||^2 -> lohx row 96, ||e_a||^2 -> alohx row 64
    embT_sq = work.tile([128, B], BF16, tag="embT_sq")
    nc.vector.tensor_mul(embT_sq, embT, embT)
    am2T_sq = work.tile([128, npc], BF16, tag="am2T_sq")
    nc.vector.tensor_mul(am2T_sq, am2T, am2T)
    psum_nn = psum1.tile([1, B], F32, tag="pnn")
    nc.tensor.matmul(psum_nn, lhsT=lhs_nn, rhs=embT_sq, start=True,
                     stop=True)
    psum_na = psum1.tile([1, B], F32, tag="pna")
    nc.tensor.matmul(psum_na, lhsT=lhs_na, rhs=am2T_sq, start=True,
                     stop=True)
    nc.scalar.copy(lohx[NNROW:NNROW + 1, :], psum_nn)
    nc.scalar.copy(alohx[NAROW:NAROW + 1, :], psum_na)

    # ---- X phase: xp = ||e_a - e_p|| + margin, xp2 = xp^2
    diff = work.tile([128, npc], BF16, tag="diff")
    nc.vector.tensor_sub(diff, a_emb, p_emb)
    dsq = work.tile([128, npc], BF16, tag="dsq")
    nc.vector.tensor_mul(dsq, diff, diff)
    xsq_cols = const.tile([128, n_tiles], F32)
    nc.vector.tensor_reduce(xsq_cols, dsq.rearrange("p (t d) -> p t d", d=D),
                            axis=mybir.AxisListType.X, op=OP.add)
    x0 = small.tile([128, n_tiles], F32, tag="x0")
    nc.scalar.activation(x0, xsq_cols, AF.Sqrt)
    xp = const.tile([128, n_tiles], F32)
    nc.vector.tensor_scalar_add(xp, x0, MARGIN)
    xp2 = const.tile([128, n_tiles], F32)
    nc.vector.tensor_mul(xp2, xp, xp)

    # ---- q matmuls, two tiles per 2-bank PSUM group
    qg = []
    for g in range(ngrp):
        gt = psq.tile([128, 2 * B], F32, tag="qg")
        qg.append(gt)
        for j in range(min(2, n_tiles - 2 * g)):
            t = 2 * g + j
            nc.tensor.matmul(gt[:, j * B:(j + 1) * B],
                             lhsT=am2T[:, bass.ts(t, 128)], rhs=embT,
                             start=True, stop=False)
    for g in range(ngrp):
        for j in range(min(2, n_tiles - 2 * g)):
            t = 2 * g + j
            nc.tensor.matmul(qg[g][:, j * B:(j + 1) * B],
                             lhsT=alohx[:, bass.ts(t, 128)], rhs=lohx,
                             start=False, stop=True)

    out_sb = small.tile([128, 2 * n_tiles], F32, tag="out_sb")

    for g in range(ngrp):
        gsz = min(2, n_tiles - 2 * g)
        Dg = work.tile([128, 2 * B], BF16, tag="Dg")
        nc.scalar.activation(Dg[:, 0:gsz * B], qg[g][:, 0:gsz * B], AF.Sqrt)
        for j in range(gsz):
            t = 2 * g + j
            Dt = Dg[:, j * B:(j + 1) * B]
            qt = qg[g][:, j * B:(j + 1) * B]
            if t < 2:
                # count on ACT: sum sign(xp^2 - q) = 2*count - 512
                scr_c = work.tile([128, B], F32, tag="scr_c")
                nc.scalar.activation(
                    scr_c, qt, AF.Sign, bias=xp2[:, t:t + 1], scale=-1.0,
                    accum_out=out_sb[:, n_tiles + t:n_tiles + t + 1])
                # loss on DVE: sum min(D - xp, 0) = -sum relu(xp - D)
                scr_l = work.tile([128, B], BF16, tag="scr_l")
                nc.vector.scalar_tensor_tensor(
                    out=scr_l, in0=Dt, scalar=xp[:, t:t + 1], in1=zeros_b,
                    op0=OP.subtract, op1=OP.min,
                    accum_out=out_sb[:, t:t + 1])
            else:
                # loss on ACT: sum relu(xp - D)
                scr_l = work.tile([128, B], BF16, tag="scr_l")
                nc.scalar.activation(
                    scr_l, Dt, AF.Relu, bias=xp[:, t:t + 1], scale=-1.0,
                    accum_out=out_sb[:, t:t + 1])
                # count on DVE: sum (D < xp)
                scr_c = work.tile([128, B], BF16, tag="scr_c")
                nc.vector.tensor_scalar(
                    scr_c, Dt, xp[:, t:t + 1], None, op0=OP.is_lt,
                    op1=OP.add,
                    accum_out=out_sb[:, n_tiles + t:n_tiles + t + 1])

    nc.sync.dma_start(out=out_d.ap(), in_=out_sb)


def _host_prepare(labels: np.ndarray, emb: np.ndarray):
    labels = np.asarray(labels).astype(np.int64)
    emb = np.ascontiguousarray(np.asarray(emb, dtype=np.float32))
    b = labels.shape[0]
    ncls = int(labels.max()) + 1
    assert ncls <= 64

    pairs_a, pairs_p = [], []
    by_class = {}
    for i, lab in enumerate(labels.tolist()):
        by_class.setdefault(lab, []).append(i)
    for idxs in by_class.values():
        for a in idxs:
            for p in idxs:
                if a != p:
                    pairs_a.append(a)
                    pairs_p.append(p)
    np_total = len(pairs_a)
    per_core = max(1, math.ceil(np_total / NCORES))
    n_tiles = max(1, math.ceil(per_core / 128))
    npc = n_tiles * 128

    m = np.bincount(labels, minlength=ncls).astype(np.int64)
    num_valid = int((m * (m - 1) * (b - m)).sum())

    sq = (emb * emb).sum(1)
    d2 = sq[:, None] + sq[None, :] - 2.0 * (emb @ emb.T)
    neq = labels[:, None] != labels[None, :]
    assert not neq.any() or d2[neq].min() > 16.0

    embT = emb.T
    onehot = (labels[None, :] ==
              np.arange(ncls)[:, None]).astype(np.float32)
    lohx = np.zeros((96, b), np.float32)
    lohx[0:ncls, :] = onehot
    lohx[NAROW, :] = 1.0          # pairs with device ||e_a||^2 row

    in_maps = []
    for k in range(NCORES):
        a_idx = pairs_a[k * per_core:(k + 1) * per_core]
        p_idx = pairs_p[k * per_core:(k + 1) * per_core]
        nreal = len(a_idx)

        am2T = np.zeros((D, npc), np.float32)
        a_emb = np.zeros((npc, D), np.float32)
        p_emb = np.zeros((npc, D), np.float32)
        alohx = np.zeros((96, npc), np.float32)
        alohx[0:ncls, :] = BIG        # pads: BIG in every class row
        if nreal:
            ga = emb[a_idx]
            am2T[:, :nreal] = (-2.0 * ga).T
            a_emb[:nreal] = ga
            p_emb[:nreal] = emb[p_idx]
            alohx[0:ncls, :nreal] = BIG * onehot[:, a_idx]

        a_emb2 = np.ascontiguousarray(
            a_emb.reshape(n_tiles, 128, D).transpose(1, 0, 2)).reshape(128, -1)
        p_emb2 = np.ascontiguousarray(
            p_emb.reshape(n_tiles, 128, D).transpose(1, 0, 2)).reshape(128, -1)
        epad = np.zeros((128, 32), np.float32)
        epad[:, 0] = 1.0              # lhs_nn ones
        epad[:, 3] = 0.25             # lhs_na quarters
        embT_t = np.concatenate([embT, epad], axis=1)
        maskc = np.concatenate([alohx, lohx], axis=1)
        ap_c = np.concatenate([a_emb2, p_emb2], axis=1)
        in_maps.append({
            "embT": np.ascontiguousarray(embT_t).astype(BF),
            "am2T": np.ascontiguousarray(am2T).astype(BF),
            "ap_emb": np.ascontiguousarray(ap_c).astype(BF),
            "mask": np.ascontiguousarray(maskc).astype(BF),
        })
    return in_maps, n_tiles, num_valid


def kernel(labels: np.ndarray, embeddings: np.ndarray):
    global LAST_RESULT
    in_maps, n_tiles, num_valid = _host_prepare(labels, embeddings)

    if n_tiles not in _PROGRAM_CACHE:
        _PROGRAM_CACHE[n_tiles] = _build_program(n_tiles)
    nc = _PROGRAM_CACHE[n_tiles]

    res = run_bass_kernel_spmd(nc, in_maps, list(range(NCORES)), trace=TRACE)
    LAST_RESULT = res

    outs = np.stack([np.asarray(r["out"], np.float64) for r in res.results])
    nact = min(2, n_tiles)
    # tiles 0..1: loss = -sum(DVE min accum), count = (sign_acc + 512)/2
    # tiles 2.. : loss = +sum(ACT relu accum), count = direct is_lt sum
    s_sum = (-outs[:, :, 0:nact].sum()) + outs[:, :, nact:n_tiles].sum()
    csign = outs[:, :, n_tiles:n_tiles + nact]
    c_sum = ((csign + 512.0) / 2.0).sum() + \
        outs[:, :, n_tiles + nact:2 * n_tiles].sum()
    loss = np.float32(s_sum / (c_sum + 1e-16))
    frac = np.float32(c_sum / (num_valid + 1e-16))
    return (np.asarray(loss, np.float32), np.asarray(frac, np.float32))
